# revision 25
# baseline (speedup 1.0000x reference)
"""DigitCapsule (dynamic routing) Trainium2 Bass kernel.

Problem: x (128,1152,8) f32, W (1,1152,10,16,8) f32 ->
  u_hat[b,r,o,do] = sum_di W[r,o,do,di] x[b,r,di]
  3 routing iterations (softmax over routes r, elementwise squash),
  output v (128,10,16,1).

Sharding: data-parallel over batch, 16 samples per core, W replicated.

Per-core layout (partition p = 16*j + b, j = r mod 8, b = batch-in-core):
  u[p, cc, do, o] = u_hat[b, 8*cc+j, o, do]   (fp16, 144 x 16 x 10 free)
u_hat is produced by 144 matmuls with a block-diagonal stationary operand
X_diag[(j,di)=64, (j',b)=128] (8 routes x 8 di contracted per matmul).
Route-sums (softmax denominator, s_j) are matmuls with a constant
delta matrix D[p,q] = (p%16 == q%16) that sums the 8 j-lanes per b and
replicates the result across all 128 partitions. The do-contraction
(agreement) is a pairwise fp16 adder tree split across DVE and GPSIMD.

Perf notes (cost-model driven):
- PE matmul costs out-cols x pe_cycle with FREE ldweights; pe_cycle ramps
  0.83 -> 0.4167 ns only after ~3us of continuous execution, so dummy
  matmuls keep PE hot across DVE-bound phases.
- DVE fp16 2-operand ops: 0.52 ns/elem; fp16 copies 0.26; reductions and
  anything touching f32/PSUM: 1.04. Act 0.833 (copies/exp). GPSIMD adds
  1.98 but runs in parallel.
- Zero-fill via broadcast tensor_copy (4x mode) + Act memzero, not Memset.
"""

import numpy as np

import concourse.bacc as bacc
import concourse.bass as bass
import concourse.tile as tile
from concourse import mybir
from concourse.bass_utils import run_bass_kernel_spmd

B, R, O, DO, DI = 128, 1152, 10, 16, 8
NCORES = 8
BC = B // NCORES          # 16 samples per core
J = 8                     # routes per matmul group
CC = R // J               # 144 matmul groups
OD = O * DO               # 160
F16 = mybir.dt.float16
F32 = mybir.dt.float32

PROD_BATCH = 3            # cc per production psum batch (1 bank, 3 cc packed)
TREE_BATCH = 24           # cc per premult/tree batch
PROD_LAG = 6              # psum batches between produce and s0-consume
POOL_CC = 4               # cc per tree-batch premult handled by GPSIMD


def _squash_chain(nc, pool, s_ps, v_out, eps):
    """v_out = squash(s_ps); s_ps is [P,16,10] f32 (PSUM).

    squash is elementwise (reference reduces over a singleton axis):
    v = s*m/((1+m)*sqrt(m+eps)), m = s^2, which is exactly s*|s|/(1+s^2)
    (eps only guards s=0, where v=0 either way). DVE-only chain - no Act
    table loads (a Sqrt<->Exp switch costs 1.28us per reload).
    """
    P = s_ps.shape[0]
    s_sb = pool.tile([P, DO, O], F32, tag="sq_s")
    m = pool.tile([P, DO, O], F32, tag="sq_m")
    a = pool.tile([P, DO, O], F32, tag="sq_a")
    d = pool.tile([P, DO, O], F32, tag="sq_d")
    rec = pool.tile([P, DO, O], F32, tag="sq_rec")
    p1 = pool.tile([P, DO, O], F32, tag="sq_p1")
    nc.vector.tensor_copy(s_sb[:], s_ps[:])
    nc.vector.tensor_mul(m[:], s_sb[:], s_sb[:])
    neg = pool.tile([P, DO, O], F32, tag="sq_n")
    nc.vector.tensor_scalar_mul(neg[:], s_sb[:], -1.0)
    nc.vector.tensor_max(a[:], s_sb[:], neg[:])
    nc.vector.tensor_scalar_add(d[:], m[:], 1.0)
    nc.vector.reciprocal(rec[:], d[:])
    nc.vector.tensor_mul(p1[:], s_sb[:], a[:])
    nc.vector.tensor_mul(v_out[:], p1[:], rec[:])


def build_nc(reps=1):
    nc = bacc.Bacc("TRN2", debug=False)
    wt_d = nc.dram_tensor("wt", [64, CC, DO, O], F16, kind="ExternalInput")
    # xp[8j+di, cc, 16j'+b] = x[b, 8cc+j, di] if j==j' else 0 (host-padded)
    xp_d = nc.dram_tensor("xp", [64, CC, 128], F16, kind="ExternalInput")
    dout_d = nc.dram_tensor("dout", [128, BC], F16, kind="ExternalInput")
    out_d = nc.dram_tensor("out", [BC, O, DO], F32, kind="ExternalOutput")

    NB = CC // PROD_BATCH     # 48 production batches
    NG = CC // TREE_BATCH     # 6 tree batches

    with tile.TileContext(nc) as tc:
        with (
            tc.tile_pool(name="const", bufs=1) as const,
            tc.tile_pool(name="prod", bufs=1) as prod,
            tc.tile_pool(name="main", bufs=1) as main,
            tc.tile_pool(name="sq", bufs=2) as sq,
            tc.tile_pool(name="tp", bufs=2) as tp,
            tc.tile_pool(name="l1p", bufs=2) as l1p,
            tc.tile_pool(name="l2p", bufs=2) as l2p,
            tc.tile_pool(name="l3p", bufs=2) as l3p,
            tc.tile_pool(name="l4p", bufs=2) as l4p,
            tc.tile_pool(name="pp", bufs=6, space=bass.MemorySpace.PSUM) as pp,
            tc.tile_pool(name="pss", bufs=1, space=bass.MemorySpace.PSUM) as pss,
            tc.tile_pool(name="psd", bufs=1, space=bass.MemorySpace.PSUM) as psd,
        ):
            eps = const.tile([128, 1], F32)
            zero = const.tile([128, 1], F32)
            nc.vector.memset(eps[:], 1e-9)
            nc.vector.memset(zero[:], 0.0)
            # preload activation tables while DMAs are in flight
            warm = const.tile([128, 1], F32)
            nc.scalar.copy(warm[:], zero[:])
            nc.scalar.activation(warm[:], zero[:],
                                 mybir.ActivationFunctionType.Exp, bias=zero[:])

            dout = const.tile([128, BC], F16)
            nc.sync.dma_start(dout[:], dout_d[:])

            # x-diag and W stream in interleaved cc-chunks so production
            # starts as soon as the first pair lands and is fed just-in-time
            wt = prod.tile([64, CC, DO, O], F16)

            d16 = const.tile([128, 128], F16)
            d16s = const.tile([128, 128], F16)
            d32 = const.tile([128, 128], F32)

            # PE warm-up: dummy matmuls ramp the p-state before production
            zstrip = const.tile([64, CC], F16)
            nc.vector.memset(zstrip[:], 0.0)
            warm_k = [0]

            def pe_warm(n, rhs=None):
                # Striped targets so WAW semaphores pipeline. `rhs` tethers
                # the dummies to live data: without a dependency the list
                # scheduler hoists them into the production phase where they
                # steal PE time instead of bridging the intended gap. Targets
                # come from the pp pool (idle outside production).
                dps = pp.tile([128, 512], F32, tag="pp")
                for _ in range(n):
                    if rhs is None:
                        s = warm_k[0] % 4
                        warm_k[0] += 1
                        nc.tensor.matmul(
                            dps[0:128, 128 * s:128 * (s + 1)],
                            zstrip[:, 0:128], zstrip[:, 0:128],
                            start=True, stop=True, skip_group_check=True)
                    else:
                        fs = rhs.free_size()
                        nst = max(1, 512 // fs)
                        s = warm_k[0] % nst
                        warm_k[0] += 1
                        nc.tensor.matmul(
                            dps[0:128, fs * s:fs * (s + 1)],
                            d16[:], rhs,
                            start=True, stop=True, skip_group_check=True)

            for _rep in range(reps):
                # ---- host-padded x block-diagonal, chunk-interleaved ----
                xd = prod.tile([64, CC, 128], F16)   # [k=(j,di), cc, col=(j,b)]
                CHUNKS = [12, 12, 24, 24, 36, 36]
                c0 = 0
                for ch in CHUNKS:
                    slc = slice(c0, c0 + ch)
                    nc.sync.dma_start(xd[:, slc, :], xp_d[:, slc, :])
                    nc.sync.dma_start(wt[:, slc], wt_d[:, slc])
                    c0 += ch

                # d16[p,q] = (p%16==q%16); d16s = d16/R (iter-0 softmax fold)
                nc.vector.tensor_copy(
                    d16[:].rearrange("p (j b) -> p j b", j=J),
                    dout[:].unsqueeze(1).broadcast_to((128, J, BC)))
                nc.vector.tensor_scalar_mul(d16s[:], d16[:], 1.0 / R)
                nc.vector.tensor_copy(d32[:], d16[:])

                pe_warm(30)

                u = main.tile([128, CC, DO, O], F16)

                # ---- produce u_hat; s0 fold interleaved on PE; copies Act/DVE
                s0 = pss.tile([128, DO, O], F32, tag="s")
                for g in range(NB + PROD_LAG):
                    if g < NB:
                        # 1 bank per tile, 3 cc packed (160 f32 each)
                        ps = pp.tile([128, 512], F32, tag="pp")
                        for i in range(PROD_BATCH):
                            cc = g * PROD_BATCH + i
                            nc.tensor.matmul(
                                ps[:, i * OD:(i + 1) * OD],
                                xd[:, cc, :], wt[:, cc, :, :],
                                start=True, stop=True,
                            )
                        sl = slice(g * PROD_BATCH, (g + 1) * PROD_BATCH)
                        srcv = ps[:, 0:3 * OD].rearrange(
                            "p (c do o) -> p c do o", c=3, do=DO)
                        # alternate copies Act/DVE (GPSIMD cannot read PSUM)
                        if g % 2 == 0:
                            nc.scalar.copy(u[:, sl, :, :], srcv)
                        else:
                            nc.vector.tensor_copy(u[:, sl, :, :], srcv)
                    if g >= PROD_LAG:
                        gs = g - PROD_LAG
                        for i in range(PROD_BATCH):
                            cc = gs * PROD_BATCH + i
                            nc.tensor.matmul(
                                s0[:], d16s[:], u[:, cc, :, :],
                                start=(cc == 0), stop=(cc == CC - 1),
                            )

                v = main.tile([128, DO, O], F16)
                _squash_chain(nc, sq, s0, v, eps)

                b16 = main.tile([128, CC, O], F16)
                e = main.tile([128, CC, O], F32)
                inv = main.tile([128, O], F32)
                c16 = main.tile([128, CC, O], F16)

                for it in (1, 2):
                    final = it == 2
                    den = psd.tile([128, O], F32, tag="den")
                    # ---- agreement: b_ij (+)= sum_do u * v  (premult + tree)
                    for g in range(NG):
                        last = g == NG - 1
                        sl = slice(g * TREE_BATCH, (g + 1) * TREE_BATCH)
                        dv = TREE_BATCH - POOL_CC
                        sld = slice(g * TREE_BATCH, g * TREE_BATCH + dv)
                        slp = slice(g * TREE_BATCH + dv, (g + 1) * TREE_BATCH)
                        t = tp.tile([128, TREE_BATCH, DO, O], F16, tag="t")
                        v_bd = v[:].unsqueeze(1).broadcast_to(
                            (128, dv, DO, O))
                        v_bp = v[:].unsqueeze(1).broadcast_to(
                            (128, POOL_CC, DO, O))
                        nc.gpsimd.tensor_mul(
                            t[:, dv:, :, :], u[:, slp, :, :], v_bp)
                        nc.vector.tensor_mul(
                            t[:, 0:dv, :, :], u[:, sld, :, :], v_bd)
                        l1 = l1p.tile([128, TREE_BATCH, 8, O], F16, tag="l1")
                        nc.vector.tensor_add(
                            l1[:], t[:, :, 0:8, :], t[:, :, 8:16, :])
                        l2 = l2p.tile([128, TREE_BATCH, 4, O], F16, tag="l2")
                        nc.vector.tensor_add(
                            l2[:], l1[:, :, 0:4, :], l1[:, :, 4:8, :])
                        # low tree levels on GPSIMD, except the last batch
                        # (DVE finishes it immediately -> short tail)
                        eng = nc.vector if last else nc.gpsimd
                        l3 = l3p.tile([128, TREE_BATCH, 2, O], F16, tag="l3")
                        eng.tensor_add(
                            l3[:], l2[:, :, 0:2, :], l2[:, :, 2:4, :])
                        if it == 1:
                            eng.tensor_add(
                                b16[:, sl, :], l3[:, :, 0, :], l3[:, :, 1, :])
                        else:
                            l4 = l4p.tile([128, TREE_BATCH, O], F16, tag="l4")
                            eng.tensor_add(l4[:], l3[:, :, 0, :], l3[:, :, 1, :])
                            eng.tensor_add(
                                b16[:, sl, :], b16[:, sl, :], l4[:])
                        # exp in f32 (no overflow, no max pass needed)
                        nc.scalar.activation(
                            e[:, sl, :], b16[:, sl, :],
                            mybir.ActivationFunctionType.Exp, bias=zero[:])
                        # keep PE hot; tether to b16 (stable tile) so the
                        # scheduler cannot hoist these nor the tp ring block
                        if not last:
                            pe_warm(11, b16[:, sl, :])
                        # softmax denominator folded on PE (idle here)
                        for i in range(TREE_BATCH):
                            cc = g * TREE_BATCH + i
                            nc.tensor.matmul(
                                den[:], d32[:], e[:, cc, :],
                                start=(cc == 0), stop=(cc == CC - 1),
                            )

                    nc.vector.reciprocal(inv[:], den[:])
                    inv_bf = inv[:].unsqueeze(1).broadcast_to((128, CC, O))
                    nc.vector.tensor_mul(c16[:], e[:], inv_bf)

                    # ---- s = sum_r c*u: c16 on Pool, premult DVE, fold PE ----
                    sp_p = BC if final else 128
                    lhs = dout[:] if final else d16[:]
                    s_ps2 = pss.tile([sp_p, DO, O], F32, tag="s")
                    for g in range(NG):
                        sl = slice(g * TREE_BATCH, (g + 1) * TREE_BATCH)
                        dv = TREE_BATCH - POOL_CC
                        sld = slice(g * TREE_BATCH, g * TREE_BATCH + dv)
                        slp = slice(g * TREE_BATCH + dv, (g + 1) * TREE_BATCH)
                        t = tp.tile([128, TREE_BATCH, DO, O], F16, tag="t")
                        c_bd = c16[:, sld, :].unsqueeze(2).broadcast_to(
                            (128, dv, DO, O))
                        c_bp = c16[:, slp, :].unsqueeze(2).broadcast_to(
                            (128, POOL_CC, DO, O))
                        nc.gpsimd.tensor_mul(
                            t[:, dv:, :, :], u[:, slp, :, :], c_bp)
                        nc.vector.tensor_mul(
                            t[:, 0:dv, :, :], u[:, sld, :, :], c_bd)
                        for i in range(TREE_BATCH):
                            cc = g * TREE_BATCH + i
                            nc.tensor.matmul(
                                s_ps2[:], lhs[:, :sp_p], t[:, i, :, :],
                                start=(cc == 0), stop=(cc == CC - 1),
                            )
                        if g < NG - 1:
                            pe_warm(8, t[:, 0, :, :])
                    if not final:
                        _squash_chain(nc, sq, s_ps2, v, eps)
                    else:
                        v2 = main.tile([BC, DO, O], F32)
                        _squash_chain(nc, sq, s_ps2, v2, eps)
                        v2p = main.tile([BC, O, DO], F32)
                        nc.vector.tensor_copy(v2p[:], v2[:].transpose((0, 2, 1)))
                        nc.sync.dma_start(out_d[:], v2p[:])

    nc.compile()
    return nc


_CACHE = {}


def _get_nc():
    if "nc" not in _CACHE:
        _CACHE["nc"] = build_nc()
    return _CACHE["nc"]


def _prep_const():
    if "const" not in _CACHE:
        p = np.arange(128)
        dout = (p[:, None] % 16 == np.arange(BC)[None, :]).astype(np.float16)
        _CACHE["const"] = dout
    return _CACHE["const"]


def kernel(x: np.ndarray, W: np.ndarray) -> np.ndarray:
    x = np.asarray(x, dtype=np.float32)
    W = np.asarray(W, dtype=np.float32)
    nc = _get_nc()
    dout = _prep_const()
    W5 = np.ascontiguousarray(W.reshape(R, O, DO, DI))
    # wt[8j+di, cc, do, o] = W[8cc+j, o, do, di]
    wt = np.ascontiguousarray(
        W5.reshape(CC, J, O, DO, DI).transpose(1, 4, 0, 3, 2)
    ).reshape(64, CC, DO, O).astype(np.float16)
    in_maps = []
    for q in range(NCORES):
        xq = x[BC * q: BC * (q + 1)]             # [16, 1152, 8]
        # xp[8j+di, cc, 16j+b] = xq[b, 8cc+j, di], zero off-diagonal
        xp = np.zeros((J, DI, CC, J, BC), dtype=np.float16)
        xv = xq.reshape(BC, CC, J, DI).transpose(2, 3, 1, 0)  # [j, di, cc, b]
        for j in range(J):
            xp[j, :, :, j, :] = xv[j]
        xp = xp.reshape(64, CC, 128)
        in_maps.append({"wt": wt, "xp": xp, "dout": dout})
    res = run_bass_kernel_spmd(nc, in_maps, core_ids=list(range(NCORES)))
    out = np.concatenate([res.results[q]["out"] for q in range(NCORES)], axis=0)
    return out.reshape(B, O, DO, 1).astype(np.float32)


# revision 26
# speedup vs baseline: 1.1099x; 1.1099x over previous
"""DigitCapsule (dynamic routing) Trainium2 Bass kernel.

Problem: x (128,1152,8) f32, W (1,1152,10,16,8) f32 ->
  u_hat[b,r,o,do] = sum_di W[r,o,do,di] x[b,r,di]
  3 routing iterations (softmax over routes r, elementwise squash),
  output v (128,10,16,1).

Sharding: data-parallel over batch, 16 samples per core, W replicated.

Per-core layout (partition p = 16*j + b, j = r mod 8, b = batch-in-core):
  u[p, cc, do, o] = u_hat[b, 8*cc+j, o, do]   (fp16, 144 x 16 x 10 free)
u_hat is produced by 144 matmuls with a block-diagonal stationary operand
X_diag[(j,di)=64, (j',b)=128] (8 routes x 8 di contracted per matmul).
Route-sums (softmax denominator, s_j) are matmuls with a constant
delta matrix D[p,q] = (p%16 == q%16) that sums the 8 j-lanes per b and
replicates the result across all 128 partitions. The do-contraction
(agreement) is a pairwise fp16 adder tree split across DVE and GPSIMD.

Perf notes (cost-model driven):
- PE matmul costs out-cols x pe_cycle with FREE ldweights; pe_cycle ramps
  0.83 -> 0.4167 ns only after ~3us of continuous execution, so dummy
  matmuls keep PE hot across DVE-bound phases.
- DVE fp16 2-operand ops: 0.52 ns/elem; fp16 copies 0.26; reductions and
  anything touching f32/PSUM: 1.04. Act 0.833 (copies/exp). GPSIMD adds
  1.98 but runs in parallel.
- Zero-fill via broadcast tensor_copy (4x mode) + Act memzero, not Memset.
"""

import numpy as np

import concourse.bacc as bacc
import concourse.bass as bass
import concourse.tile as tile
from concourse import mybir
from concourse.bass_utils import run_bass_kernel_spmd

B, R, O, DO, DI = 128, 1152, 10, 16, 8
NCORES = 8
BC = B // NCORES          # 16 samples per core
J = 8                     # routes per matmul group
CC = R // J               # 144 matmul groups
OD = O * DO               # 160
F16 = mybir.dt.float16
F32 = mybir.dt.float32

PROD_BATCH = 3            # cc per production psum batch (1 bank, 3 cc packed)
TREE_BATCH = 24           # cc per premult/tree batch
PROD_LAG = 6              # psum batches between produce and s0-consume
POOL_CC = 4               # cc per tree-batch premult handled by GPSIMD


def _squash_chain(nc, pool, s_ps, v_out, eps):
    """v_out = squash(s_ps); s_ps is [P,16,10] f32 (PSUM).

    squash is elementwise (reference reduces over a singleton axis):
    v = s*m/((1+m)*sqrt(m+eps)), m = s^2, which is exactly s*|s|/(1+s^2)
    (eps only guards s=0, where v=0 either way). DVE-only chain - no Act
    table loads (a Sqrt<->Exp switch costs 1.28us per reload).
    """
    P = s_ps.shape[0]
    s_sb = pool.tile([P, DO, O], F32, tag="sq_s")
    m = pool.tile([P, DO, O], F32, tag="sq_m")
    a = pool.tile([P, DO, O], F32, tag="sq_a")
    d = pool.tile([P, DO, O], F32, tag="sq_d")
    rec = pool.tile([P, DO, O], F32, tag="sq_rec")
    p1 = pool.tile([P, DO, O], F32, tag="sq_p1")
    nc.vector.tensor_copy(s_sb[:], s_ps[:])
    nc.vector.tensor_mul(m[:], s_sb[:], s_sb[:])
    neg = pool.tile([P, DO, O], F32, tag="sq_n")
    nc.vector.tensor_scalar_mul(neg[:], s_sb[:], -1.0)
    nc.vector.tensor_max(a[:], s_sb[:], neg[:])
    nc.vector.tensor_scalar_add(d[:], m[:], 1.0)
    nc.vector.reciprocal(rec[:], d[:])
    nc.vector.tensor_mul(p1[:], s_sb[:], a[:])
    nc.vector.tensor_mul(v_out[:], p1[:], rec[:])


def build_nc(reps=1):
    nc = bacc.Bacc("TRN2", debug=False)
    wt_d = nc.dram_tensor("wt", [64, CC, DO, O], F16, kind="ExternalInput")
    # xp[8j+di, cc, 16j'+b] = x[b, 8cc+j, di] if j==j' else 0 (host-padded)
    xp_d = nc.dram_tensor("xp", [64, CC, 128], F16, kind="ExternalInput")
    dout_d = nc.dram_tensor("dout", [128, BC], F16, kind="ExternalInput")
    out_d = nc.dram_tensor("out", [BC, O, DO], F32, kind="ExternalOutput")

    NB = CC // PROD_BATCH     # 48 production batches
    NG = CC // TREE_BATCH     # 6 tree batches

    with tile.TileContext(nc) as tc:
        with (
            tc.tile_pool(name="const", bufs=1) as const,
            tc.tile_pool(name="prod", bufs=1) as prod,
            tc.tile_pool(name="main", bufs=1) as main,
            tc.tile_pool(name="sq", bufs=2) as sq,
            tc.tile_pool(name="tp", bufs=2) as tp,
            tc.tile_pool(name="l1p", bufs=2) as l1p,
            tc.tile_pool(name="l2p", bufs=2) as l2p,
            tc.tile_pool(name="l3p", bufs=2) as l3p,
            tc.tile_pool(name="l4p", bufs=2) as l4p,
            tc.tile_pool(name="pp", bufs=6, space=bass.MemorySpace.PSUM) as pp,
            tc.tile_pool(name="pss", bufs=1, space=bass.MemorySpace.PSUM) as pss,
            tc.tile_pool(name="psd", bufs=1, space=bass.MemorySpace.PSUM) as psd,
        ):
            eps = const.tile([128, 1], F32)
            zero = const.tile([128, 1], F32)
            nc.vector.memset(eps[:], 1e-9)
            nc.vector.memset(zero[:], 0.0)
            # preload activation tables while DMAs are in flight
            warm = const.tile([128, 1], F32)
            nc.scalar.copy(warm[:], zero[:])
            nc.scalar.activation(warm[:], zero[:],
                                 mybir.ActivationFunctionType.Exp, bias=zero[:])

            dout = const.tile([128, BC], F16)
            nc.sync.dma_start(dout[:], dout_d[:])

            # x-diag and W stream in interleaved cc-chunks so production
            # starts as soon as the first pair lands and is fed just-in-time
            wt = prod.tile([64, CC, DO, O], F16)

            d16 = const.tile([128, 128], F16)
            d16s = const.tile([128, 128], F16)
            d32 = const.tile([128, 128], F32)

            # PE warm-up: dummy matmuls ramp the p-state before production
            zstrip = const.tile([64, CC], F16)
            nc.vector.memset(zstrip[:], 0.0)
            warm_k = [0]

            def pe_warm(n, rhs=None):
                # Striped targets so WAW semaphores pipeline. `rhs` tethers
                # the dummies to live data: without a dependency the list
                # scheduler hoists them into the production phase where they
                # steal PE time instead of bridging the intended gap. Targets
                # come from the pp pool (idle outside production).
                dps = pp.tile([128, 512], F32, tag="pp")
                for _ in range(n):
                    if rhs is None:
                        s = warm_k[0] % 4
                        warm_k[0] += 1
                        nc.tensor.matmul(
                            dps[0:128, 128 * s:128 * (s + 1)],
                            zstrip[:, 0:128], zstrip[:, 0:128],
                            start=True, stop=True, skip_group_check=True)
                    else:
                        fs = rhs.free_size()
                        nst = max(1, 512 // fs)
                        s = warm_k[0] % nst
                        warm_k[0] += 1
                        nc.tensor.matmul(
                            dps[0:128, fs * s:fs * (s + 1)],
                            d16[:], rhs,
                            start=True, stop=True, skip_group_check=True)

            for _rep in range(reps):
                # ---- host-padded x block-diagonal, chunk-interleaved ----
                xd = prod.tile([64, CC, 128], F16)   # [k=(j,di), cc, col=(j,b)]
                CHUNKS = [12, 12, 24, 24, 36, 36]
                c0 = 0
                for ch in CHUNKS:
                    slc = slice(c0, c0 + ch)
                    nc.sync.dma_start(xd[:, slc, :], xp_d[:, slc, :])
                    nc.sync.dma_start(wt[:, slc], wt_d[:, slc])
                    c0 += ch

                # d16[p,q] = (p%16==q%16); d16s = d16/R (iter-0 softmax fold)
                nc.vector.tensor_copy(
                    d16[:].rearrange("p (j b) -> p j b", j=J),
                    dout[:].unsqueeze(1).broadcast_to((128, J, BC)))
                nc.vector.tensor_scalar_mul(d16s[:], d16[:], 1.0 / R)
                nc.vector.tensor_copy(d32[:], d16[:])

                pe_warm(30)

                u = main.tile([128, CC, DO, O], F16)

                # ---- produce u_hat; s0 fold interleaved on PE; copies Act/DVE
                s0 = pss.tile([128, DO, O], F32, tag="s")
                for g in range(NB + PROD_LAG):
                    if g < NB:
                        # 1 bank per tile, 3 cc packed (160 f32 each)
                        ps = pp.tile([128, 512], F32, tag="pp")
                        for i in range(PROD_BATCH):
                            cc = g * PROD_BATCH + i
                            nc.tensor.matmul(
                                ps[:, i * OD:(i + 1) * OD],
                                xd[:, cc, :], wt[:, cc, :, :],
                                start=True, stop=True,
                            )
                        sl = slice(g * PROD_BATCH, (g + 1) * PROD_BATCH)
                        srcv = ps[:, 0:3 * OD].rearrange(
                            "p (c do o) -> p c do o", c=3, do=DO)
                        # alternate copies Act/DVE (GPSIMD cannot read PSUM)
                        if g % 2 == 0:
                            nc.scalar.copy(u[:, sl, :, :], srcv)
                        else:
                            nc.vector.tensor_copy(u[:, sl, :, :], srcv)
                    if g >= PROD_LAG:
                        gs = g - PROD_LAG
                        for i in range(PROD_BATCH):
                            cc = gs * PROD_BATCH + i
                            nc.tensor.matmul(
                                s0[:], d16s[:], u[:, cc, :, :],
                                start=(cc == 0), stop=(cc == CC - 1),
                            )

                v = main.tile([128, DO, O], F16)
                _squash_chain(nc, sq, s0, v, eps)

                b16 = main.tile([128, CC, O], F16)
                e = main.tile([128, CC, O], F32)
                inv = main.tile([128, O], F32)
                c16 = main.tile([128, CC, O], F16)

                for it in (1, 2):
                    final = it == 2
                    den = psd.tile([128, O], F32, tag="den")
                    # ---- agreement: b_ij (+)= sum_do u * v  (premult + tree)
                    # batches: 5 full tree batches + two half batches at the
                    # end (shorter b -> exp -> den -> inv serial tail)
                    asl = [(g * TREE_BATCH, TREE_BATCH) for g in range(NG - 1)]
                    asl += [((NG - 1) * TREE_BATCH, 12),
                            ((NG - 1) * TREE_BATCH + 12, 12)]
                    for g, (a0, alen) in enumerate(asl):
                        last = g >= NG - 1
                        sl = slice(a0, a0 + alen)
                        dv = alen - (0 if last else POOL_CC)
                        sld = slice(a0, a0 + dv)
                        slp = slice(a0 + dv, a0 + alen)
                        t = tp.tile([128, TREE_BATCH, DO, O], F16, tag="t")
                        v_bd = v[:].unsqueeze(1).broadcast_to(
                            (128, dv, DO, O))
                        if not last:
                            v_bp = v[:].unsqueeze(1).broadcast_to(
                                (128, POOL_CC, DO, O))
                            nc.gpsimd.tensor_mul(
                                t[:, dv:alen, :, :], u[:, slp, :, :], v_bp)
                        nc.vector.tensor_mul(
                            t[:, 0:dv, :, :], u[:, sld, :, :], v_bd)
                        l1 = l1p.tile([128, TREE_BATCH, 8, O], F16, tag="l1")
                        nc.vector.tensor_add(
                            l1[:, 0:alen], t[:, 0:alen, 0:8, :],
                            t[:, 0:alen, 8:16, :])
                        l2 = l2p.tile([128, TREE_BATCH, 4, O], F16, tag="l2")
                        nc.vector.tensor_add(
                            l2[:, 0:alen], l1[:, 0:alen, 0:4, :],
                            l1[:, 0:alen, 4:8, :])
                        # low tree levels on GPSIMD, except the last batches
                        # (DVE finishes them immediately -> short tail)
                        eng = nc.vector if last else nc.gpsimd
                        l3 = l3p.tile([128, TREE_BATCH, 2, O], F16, tag="l3")
                        eng.tensor_add(
                            l3[:, 0:alen], l2[:, 0:alen, 0:2, :],
                            l2[:, 0:alen, 2:4, :])
                        if it == 1:
                            eng.tensor_add(
                                b16[:, sl, :], l3[:, 0:alen, 0, :],
                                l3[:, 0:alen, 1, :])
                        else:
                            l4 = l4p.tile([128, TREE_BATCH, O], F16, tag="l4")
                            eng.tensor_add(l4[:, 0:alen], l3[:, 0:alen, 0, :],
                                           l3[:, 0:alen, 1, :])
                            eng.tensor_add(
                                b16[:, sl, :], b16[:, sl, :], l4[:, 0:alen])
                        # exp in f32 (no overflow, no max pass needed)
                        nc.scalar.activation(
                            e[:, sl, :], b16[:, sl, :],
                            mybir.ActivationFunctionType.Exp, bias=zero[:])
                        # keep PE hot; tether to b16 (stable tile) so the
                        # scheduler cannot hoist these nor the tp ring block
                        if not last:
                            pe_warm(8, b16[:, sl, :])
                        # softmax denominator folded on PE (idle here)
                        for i in range(alen):
                            cc = a0 + i
                            nc.tensor.matmul(
                                den[:], d32[:], e[:, cc, :],
                                start=(cc == 0), stop=(cc == CC - 1),
                            )

                    nc.vector.reciprocal(inv[:], den[:])

                    # ---- s = sum_r c*u: c16 on Pool, premult DVE, fold PE ----
                    sp_p = BC if final else 128
                    lhs = dout[:] if final else d16[:]
                    s_ps2 = pss.tile([sp_p, DO, O], F32, tag="s")
                    for g in range(NG):
                        sl = slice(g * TREE_BATCH, (g + 1) * TREE_BATCH)
                        dv = TREE_BATCH - POOL_CC
                        sld = slice(g * TREE_BATCH, g * TREE_BATCH + dv)
                        slp = slice(g * TREE_BATCH + dv, (g + 1) * TREE_BATCH)
                        inv_b = inv[:].unsqueeze(1).broadcast_to(
                            (128, TREE_BATCH, O))
                        ceng = nc.vector if g == 0 else nc.gpsimd
                        ceng.tensor_mul(c16[:, sl, :], e[:, sl, :], inv_b)
                        t = tp.tile([128, TREE_BATCH, DO, O], F16, tag="t")
                        c_bd = c16[:, sld, :].unsqueeze(2).broadcast_to(
                            (128, dv, DO, O))
                        c_bp = c16[:, slp, :].unsqueeze(2).broadcast_to(
                            (128, POOL_CC, DO, O))
                        nc.gpsimd.tensor_mul(
                            t[:, dv:, :, :], u[:, slp, :, :], c_bp)
                        nc.vector.tensor_mul(
                            t[:, 0:dv, :, :], u[:, sld, :, :], c_bd)
                        for i in range(TREE_BATCH):
                            cc = g * TREE_BATCH + i
                            nc.tensor.matmul(
                                s_ps2[:], lhs[:, :sp_p], t[:, i, :, :],
                                start=(cc == 0), stop=(cc == CC - 1),
                            )
                        if g < NG - 1:
                            pe_warm(8, t[:, 0, :, :])
                    if not final:
                        _squash_chain(nc, sq, s_ps2, v, eps)
                    else:
                        v2 = main.tile([BC, DO, O], F32)
                        _squash_chain(nc, sq, s_ps2, v2, eps)
                        v2p = main.tile([BC, O, DO], F32)
                        nc.vector.tensor_copy(v2p[:], v2[:].transpose((0, 2, 1)))
                        nc.sync.dma_start(out_d[:], v2p[:])

    nc.compile()
    return nc


_CACHE = {}


def _get_nc():
    if "nc" not in _CACHE:
        _CACHE["nc"] = build_nc()
    return _CACHE["nc"]


def _prep_const():
    if "const" not in _CACHE:
        p = np.arange(128)
        dout = (p[:, None] % 16 == np.arange(BC)[None, :]).astype(np.float16)
        _CACHE["const"] = dout
    return _CACHE["const"]


def kernel(x: np.ndarray, W: np.ndarray) -> np.ndarray:
    x = np.asarray(x, dtype=np.float32)
    W = np.asarray(W, dtype=np.float32)
    nc = _get_nc()
    dout = _prep_const()
    W5 = np.ascontiguousarray(W.reshape(R, O, DO, DI))
    # wt[8j+di, cc, do, o] = W[8cc+j, o, do, di]
    wt = np.ascontiguousarray(
        W5.reshape(CC, J, O, DO, DI).transpose(1, 4, 0, 3, 2)
    ).reshape(64, CC, DO, O).astype(np.float16)
    in_maps = []
    for q in range(NCORES):
        xq = x[BC * q: BC * (q + 1)]             # [16, 1152, 8]
        # xp[8j+di, cc, 16j+b] = xq[b, 8cc+j, di], zero off-diagonal
        xp = np.zeros((J, DI, CC, J, BC), dtype=np.float16)
        xv = xq.reshape(BC, CC, J, DI).transpose(2, 3, 1, 0)  # [j, di, cc, b]
        for j in range(J):
            xp[j, :, :, j, :] = xv[j]
        xp = xp.reshape(64, CC, 128)
        in_maps.append({"wt": wt, "xp": xp, "dout": dout})
    res = run_bass_kernel_spmd(nc, in_maps, core_ids=list(range(NCORES)))
    out = np.concatenate([res.results[q]["out"] for q in range(NCORES)], axis=0)
    return out.reshape(B, O, DO, 1).astype(np.float32)
